# revision 54
# baseline (speedup 1.0000x reference)
"""Trainium2 Bass kernel for nn_Attention_52012053955205.

Multi-head causal attention, B=2 S=2048 D=1024 H=16 HD=64, fp32.

Sharding: 8 cores = 2-way batch x 4-way heads. Each core computes, for its
batch item b and its 4 heads, the partial output sum_h z_h @ W_O_h  as a
full [S, D] tile; the host sums the 4 partials per batch and adds b_O.

Per-core dataflow (everything "transposed" so the softmax denominator is a
free by-product of matmuls):
  xT [D, S] (host-pretransposed) -> QT/KT [d_pair=128, S] via projection
  matmuls (W packed per head-pair, 1/sqrt(HD) folded into W_Q host-side;
  b_Q/b_K added via the ACT-copy per-partition bias).
  V [s, 4*64] natural layout, b_V added via a DVE add with a
  partition-broadcast bias tile; a ones column is appended per head
  (V' [s, 65]) so the z-matmul also produces the softmax denominator.
  Scores TRANSPOSED: S_T[k_tile, q] = KT_tile.T @ QT_block -- both heads
  of a pair as two K=64 matmuls into one 2-bank PSUM tile (disjoint PE
  row groups run concurrently). One [128, 1024] exp on ScalarE per k-tile
  (no max-subtraction: scores are bounded, exp fits fp32); causal masking
  is a post-exp 0/1 multiply on diagonal tiles, off the ACT critical path.
  z_unnorm^T [65, q] accumulated over k tiles in PSUM (row 64 = denom).
  Normalization: reciprocal of denom row, broadcast to 128 partitions via
  K=1 matmuls against selector rows, one DVE multiply per head.
  Output projection: out[s, D] = znorm_pair^T.T @ W_O_pair, accumulated
  over the two head pairs in PSUM; DMA'd out contiguously.
  Matmuls run as float32r (TF32, 1 cycle/row vs 4 for fp32); host
  pre-rounds inputs to TF32 (RNE). End-to-end rel err ~3.6e-4.

  Scheduling (order=3, the default): a fine-grained software pipeline.
  Attention for q-block Qb runs a zpipe-depth-2 kt loop (z matmuls lag
  scores by 2 so the PE never sits on the exp's latency); projection
  quanta for block sb+1 and out-projection quanta for block sb-1 are
  paced INTO the kt loop as PE filler. Softmax normalization is split:
  1/denom = exp(-ln denom) on ScalarE (both funcs share one ACT table;
  InstReciprocal on a [1,512] tile costs 3.3us on DVE) and the
  unnormalized z is staged to SBUF at pair end (frees the psZ PSUM
  banks); the broadcast (K=1 matmuls against selector rows) and the
  in-place normalize multiplies are DEFERRED and injected into the next
  pair's kt loop so the PE never stalls on the reciprocal. Causal
  masking is a post-exp 0/1 multiply trimmed to the columns that
  actually contain masked elements. Startup streams weights on the ACT
  DMA queue and x on the SP queue in parallel, and attention(0) begins
  right after pair0's Q/K with V/pair1-proj pulled in as early filler.
"""

import json

import numpy as np

B, S, D, H, HD = 2, 2048, 1024, 16, 64
NCORES = 8
HPC = 4  # heads per core

_STATE = {}


# ---------------------------------------------------------------------------
# Tile tail-drain workaround: walrus in this container rejects >2 sem waits
# on one instruction ("Too many sync wait commands"). Split the tail waits
# across one sync NOP per logical proc; the drain itself then needs none.
# ---------------------------------------------------------------------------
def _patch_ldw_opt():
    """Recompile with walrus's LDWEIGHTS pipelining enabled: the default
    --enable-ldw-opt=false leaves each matmul's weight load serialized with
    the previous matmul on the PE (~+190ns per matmul on this kernel)."""
    import concourse.bass_utils as bu

    if getattr(bu, "_ldw_opt_patch", False):
        return
    orig = bu.run_command

    def patched(argv, **kwargs):
        if getattr(bu, "_ldw_opt_on", False):
            argv = [
                "--enable-ldw-opt=true" if a == "--enable-ldw-opt=false" else a
                for a in argv
            ]
        return orig(argv, **kwargs)

    bu.run_command = patched
    bu._ldw_opt_patch = True


def _patch_tile_drain():
    import concourse.tile as tile
    from concourse.vector_clock import ScopedClock, VectorClock

    if getattr(tile.TileContext, "_drain_split_patch", False):
        return

    def _split_drain_and_barrier(self, tick_clock, wait_clock):
        gc = tick_clock.global_clock
        n = len(gc)
        for proc in range(n):
            t = gc[proc]
            if t > 0:
                vc = VectorClock([t if i == proc else 0 for i in range(n)])
                nop = self.nc.sync.nop(nofuse=True)
                wait_clock.add_sem_waits(nop.ins, ScopedClock({None: vc}))
        self.nc.sync.drain()
        self.nc.all_engine_barrier()
        assert self.sems is not None
        popped = self.nc._tile_sem_poison_stack.pop()
        assert popped is self._sem_poison
        self.nc.clear_and_free_semaphores(list(self.sems.allocated().values()))
        self.nc.all_engine_barrier()

    tile.TileContext._drain_and_barrier = _split_drain_and_barrier
    tile.TileContext._drain_split_patch = True


def _split_waits_bir(bir: bytes) -> bytes:
    """Walrus in this container allows only one sem wait per instruction.
    Spill extra on_wait entries onto same-engine NoOps inserted right
    before the instruction (the NX executes them in stream order)."""
    d = json.loads(bir)
    ctr = 0
    for f in d["functions"]:
        for bb in f["blocks"]:
            new = []
            for ins in bb["instructions"]:
                si = ins.get("sync_info")
                waits = si.get("on_wait", []) if si else []
                if len(waits) > 1:
                    for w in waits[:-1]:
                        ctr += 1
                        new.append(
                            {
                                "debug": ins.get("debug", 0),
                                "engine": ins["engine"],
                                "ins": [],
                                "name": f"I-wsplit-{ctr}",
                                "opcode": "NoOp",
                                "outs": [],
                                "sync_info": {"on_update": [], "on_wait": [w]},
                            }
                        )
                    si["on_wait"] = [waits[-1]]
                new.append(ins)
            bb["instructions"] = new
    return json.dumps(d).encode()


def _hook_wait_split(nc):
    orig = nc.to_json_bytes

    def patched():
        return _split_waits_bir(orig())

    nc.to_json_bytes = patched
    return nc


# ---------------------------------------------------------------------------
# Bass program (identical on all 8 cores; all per-core data arrives as
# ExternalInputs)
# ---------------------------------------------------------------------------
def _build_nc(reps=1, f32r=True, qk_bias=True, v_bias=True, upto=3, order=0, zpipe=1, xpf=1, xsplit=4, bcast_dma=0, warmup=0, recip_ln=1, ldwopt=1, sel2=0, pace=4, gmask=0):
    import concourse.bass as bass
    import concourse.mybir as mybir
    import concourse.tile as tile

    FP = mybir.dt.float32
    FR = mybir.dt.float32r
    AF = mybir.ActivationFunctionType
    _patch_tile_drain()
    _patch_ldw_opt()
    import concourse.bass_utils as _bu

    _bu._ldw_opt_on = bool(ldwopt)

    nc = bass.Bass(target_bir_lowering=False)

    MT = FR if f32r else FP  # dtype for every matmul operand tile

    mm = nc.tensor.matmul

    xT = nc.dram_tensor("xt", [D, S], MT, kind="ExternalInput")
    wq = nc.dram_tensor("wq", [2, D, 128], MT, kind="ExternalInput")
    wk = nc.dram_tensor("wk", [2, D, 128], MT, kind="ExternalInput")
    wv = nc.dram_tensor("wv", [D, 256], MT, kind="ExternalInput")
    wo = nc.dram_tensor("wo", [2, 128, D], MT, kind="ExternalInput")
    bq = nc.dram_tensor("bq", [2, 128], FP, kind="ExternalInput")
    bk = nc.dram_tensor("bk", [2, 128], FP, kind="ExternalInput")
    bv = nc.dram_tensor("bv", [256], FP, kind="ExternalInput")
    masks = nc.dram_tensor("masks", [4, 128, 512], MT, kind="ExternalInput")
    sel = nc.dram_tensor("sel", [2, 128], MT, kind="ExternalInput")
    out = nc.dram_tensor("out", [S, D], FP, kind="ExternalOutput")
    rdbc = (
        nc.dram_tensor("rdbc", [16, 512], MT, kind="ExternalOutput")
        if bcast_dma
        else None
    )

    with tile.TileContext(nc) as tc:
        with (
            nc.allow_low_precision(reason="tf32 (fp32r) matmul pipeline"),
            tc.tile_pool(name="consts", bufs=1) as consts,
            tc.tile_pool(name="xp", bufs=3) as xp,
            tc.tile_pool(name="qk", bufs=1) as qk,
            tc.tile_pool(name="vp", bufs=1) as vp,
            tc.tile_pool(name="zp", bufs=1) as zp,
            tc.tile_pool(name="etp", bufs=5) as etp,
            tc.tile_pool(name="bcp", bufs=2) as bcp,
            tc.tile_pool(name="rdpool", bufs=6) as rdpool,
            tc.tile_pool(name="ostp", bufs=2) as ostp,
            tc.tile_pool(name="psA", bufs=2, space="PSUM") as psA,
            tc.tile_pool(name="psB", bufs=(2 if order == 3 else 4), space="PSUM") as psB,
            tc.tile_pool(name="psP", bufs=2, space="PSUM") as psP,
        ):
            # ---- constants ----
            # group A: needed by the first projections -- DMA'd first,
            # fine-grained so the first proj matmul starts ~2.5us in rather
            # than waiting for the whole 3MB startup block
            xTr = xT[:].rearrange("(c p) s -> p c s", p=128)
            if warmup:
                # ramp the PE p-state during the startup DMA wait: memset a
                # junk tile (no DMA dep) and run throwaway matmuls on it
                wu_t = consts.tile([128, 512], MT, tag="wu")
                nc.vector.memset(wu_t[:, :].bitcast(FP), 1.0)
                psW = psA.tile([128, 2, 512], FP, tag="A", name="warmup")
                for i in range(warmup):
                    mm(
                        psW[:, i % 2, :],
                        wu_t[:, 0:128],
                        wu_t,
                        start=True,
                        stop=True,
                    )
            wq_sb = consts.tile([128, 2, 8, 128], MT, tag="wq")
            wk_sb = consts.tile([128, 2, 8, 128], MT, tag="wk")
            wqr = wq[:].rearrange("a (c p) d -> p a c d", p=128)
            wkr = wk[:].rearrange("a (c p) d -> p a c d", p=128)
            x0_t = xp.tile([128, 8, 512], MT, tag="x", name="x_pre0")
            # weights stream from the ACT queue (idle at start), x from SP:
            # the two DGE sequencers work in parallel
            nc.scalar.dma_start(out=wq_sb[:, 0], in_=wqr[:, 0])
            nc.sync.dma_start(out=x0_t[:, 0:4, :], in_=xTr[:, 0:4, 0:512])
            nc.scalar.dma_start(out=wk_sb[:, 0], in_=wkr[:, 0])
            nc.sync.dma_start(out=x0_t[:, 4:8, :], in_=xTr[:, 4:8, 0:512])
            nc.scalar.dma_start(out=wq_sb[:, 1], in_=wqr[:, 1])
            nc.scalar.dma_start(out=wk_sb[:, 1], in_=wkr[:, 1])
            wv_sb = consts.tile([128, 8, 256], MT, tag="wv")
            nc.scalar.dma_start(
                out=wv_sb, in_=wv[:].rearrange("(c p) d -> p c d", p=128)
            )
            if qk_bias:
                bq_sb = consts.tile([128, 2], FP, tag="bq")
                nc.sync.dma_start(out=bq_sb, in_=bq[:].rearrange("a p -> p a"))
                bk_sb = consts.tile([128, 2], FP, tag="bk")
                nc.sync.dma_start(out=bk_sb, in_=bk[:].rearrange("a p -> p a"))
            if v_bias:
                bvbc_sb = consts.tile([128, 4, 64], FP, tag="bvbc")
                nc.sync.dma_start(
                    out=bvbc_sb,
                    in_=bass.AP(tensor=bv, offset=0, ap=[[0, 128], [1, 256]]),
                )
            # group B: not needed until attention / out-proj of the first
            # s-block -- emitted lazily below
            wo_sb = consts.tile([128, 2, D], MT, tag="wo")
            masks_sb = consts.tile([128, 4, 512], MT, tag="masks")
            sel_sb = consts.tile([1, 2, 128], MT, tag="sel")
            sel2_sb = consts.tile([2, 128], MT, tag="sel2")

            def emit_const_group_b():
                nc.sync.dma_start(
                    out=masks_sb, in_=masks[:].rearrange("m p j -> p m j")
                )
                nc.sync.dma_start(
                    out=sel_sb,
                    in_=bass.AP(
                        tensor=sel, offset=0, ap=[[256, 1], [128, 2], [1, 128]]
                    ),
                )
                nc.sync.dma_start(out=sel2_sb, in_=sel[:])
                nc.sync.dma_start(
                    out=wo_sb, in_=wo[:].rearrange("a p d -> p a d")
                )

            def emit_x(sb, x_pre=None):
                if x_pre is not None:
                    return x_pre
                x_t = xp.tile([128, 8, 512], MT, tag="x", name=f"x_{sb}")
                w = 8 // xsplit
                for dd in range(xsplit):
                    nc.sync.dma_start(
                        out=x_t[:, w * dd : w * dd + w, :],
                        in_=xTr[
                            :, w * dd : w * dd + w, sb * 512 : (sb + 1) * 512
                        ],
                    )
                return x_t

            def emit_qk(sb, x_t, qt_sb, kt_sb, pairs=(0, 1)):
                for pair in pairs:
                    psQ = psA.tile([128, 2, 512], FP, tag="A", name=f"psQ_{sb}_{pair}")
                    for c in range(8):
                        mm(
                            psQ[:, 0, :],
                            wq_sb[:, pair, c, :],
                            x_t[:, c, :],
                            start=(c == 0),
                            stop=(c == 7),
                        )
                    for c in range(8):
                        mm(
                            psQ[:, 1, :],
                            wk_sb[:, pair, c, :],
                            x_t[:, c, :],
                            start=(c == 0),
                            stop=(c == 7),
                        )
                    qt_dst = qt_sb[:, pair, sb * 512 : (sb + 1) * 512]
                    kt_dst = kt_sb[:, pair, sb * 512 : (sb + 1) * 512]
                    if qk_bias:
                        nc.scalar.activation(
                            qt_dst, psQ[:, 0, :], AF.Identity,
                            bias=bq_sb[:, pair : pair + 1],
                        )
                        nc.scalar.activation(
                            kt_dst, psQ[:, 1, :], AF.Identity,
                            bias=bk_sb[:, pair : pair + 1],
                        )
                    elif pair == 0:
                        nc.vector.tensor_copy(qt_dst, psQ[:, 0, :])
                        nc.vector.tensor_copy(kt_dst, psQ[:, 1, :])
                    else:
                        nc.scalar.activation(qt_dst, psQ[:, 0, :], AF.Copy)
                        nc.scalar.activation(kt_dst, psQ[:, 1, :], AF.Copy)

            def emit_v(sb, x_t, v_sb):
                for stl in range(4):
                    st = sb * 4 + stl
                    psV = psB.tile([128, 256], FP, tag="ZB", name=f"psV_{st}")
                    for c in range(8):
                        mm(
                            psV,
                            x_t[:, c, stl * 128 : (stl + 1) * 128],
                            wv_sb[:, c, :],
                            start=(c == 0),
                            stop=(c == 7),
                        )
                    if v_bias:
                        nc.vector.tensor_add(
                            v_sb[:, st, :, 0:64],
                            psV.rearrange("p (h d) -> p h d", h=4),
                            bvbc_sb,
                        )
                    else:
                        # zero b_V: plain PSUM->SBUF move, on ACT (the proj
                        # phase has ACT headroom; DVE is the loaded engine)
                        nc.scalar.activation(
                            v_sb[:, st, :, 0:64],
                            psV.rearrange("p (h d) -> p h d", h=4),
                            AF.Copy,
                        )

            def emit_attn_both(Qb, qt_sb, kt_sb, v_sb, znp):
                """Both head pairs' attention for one q-block, kt loops
                interleaved: two independent score->exp->z chains keep PE
                fed while either waits on ScalarE."""
                q0, q1 = Qb * 512, (Qb + 1) * 512
                ktmax = 4 * (Qb + 1)
                psZs = {}
                for pair in range(2):
                    for hh in range(2):
                        psZs[(pair, hh)] = psB.tile(
                            [65, 512], FP, tag="ZB",
                            name=f"psZ_{pair}_{Qb}_{hh}",
                        )
                for kt in range(ktmax):
                    diag = kt >= 4 * Qb
                    r = (kt - 4 * Qb) * 128 if diag else 0
                    r = min(r, 256)
                    for pair in range(2):
                        psS = psA.tile(
                            [128, 2, 512], FP, tag="A",
                            name=f"psS_{pair}_{Qb}_{kt}",
                        )
                        for hh in range(2):
                            po = hh * 64
                            mm(
                                psS[:, hh, r:512],
                                kt_sb[po : po + 64, pair, kt * 128 : (kt + 1) * 128],
                                qt_sb[po : po + 64, pair, q0 + r : q1],
                                start=True,
                                stop=True,
                            )
                        e_t = etp.tile(
                            [128, 2, 512], MT, tag="et",
                            name=f"et_{pair}_{Qb}_{kt}",
                        )
                        nc.scalar.activation(
                            e_t[:, :, r:512], psS[:, :, r:512], AF.Exp
                        )
                        if diag:
                            for hh in range(2):
                                nc.vector.tensor_mul(
                                    e_t[:, hh, r:512],
                                    e_t[:, hh, r:512],
                                    masks_sb[:, kt % 4, r:512],
                                )
                        for hh in range(2):
                            mm(
                                psZs[(pair, hh)][:, r:512],
                                v_sb[:, kt, 2 * pair + hh, :],
                                e_t[:, hh, r:512],
                                start=(kt == 0),
                                stop=(kt == ktmax - 1),
                            )
                for pair in range(2):
                    rds = []
                    for hh in range(2):
                        rd_h = rdpool.tile(
                            [1, 512], MT, tag="rd", name=f"rd_{pair}_{Qb}_{hh}"
                        )
                        rds.append(rd_h)
                        nc.vector.reciprocal(rd_h, psZs[(pair, hh)][64:65, :])
                    bc = psA.tile([128, 512], FP, tag="A", name=f"bc_{pair}_{Qb}")
                    mm(bc, sel_sb[:, 0, :], rds[0], start=True, stop=False)
                    mm(bc, sel_sb[:, 1, :], rds[1], start=False, stop=True)
                    bcs = bcp.tile(
                        [128, 512], FP, tag="bcs", name=f"bcs_{pair}_{Qb}"
                    )
                    nc.vector.tensor_copy(bcs, bc)
                    nc.vector.tensor_mul(
                        znp[0:64, pair, Qb, :],
                        psZs[(pair, 0)][0:64, :],
                        bcs[0:64, :],
                    )
                    zc = bcp.tile([128, 512], FP, tag="zc", name=f"zc_{pair}_{Qb}")
                    nc.vector.tensor_copy(zc[64:128, :], psZs[(pair, 1)][0:64, :])
                    nc.vector.tensor_mul(
                        znp[64:128, pair, Qb, :],
                        zc[64:128, :],
                        bcs[64:128, :],
                    )

            def emit_attn(pair, Qb, qt_sb, kt_sb, v_sb, znp):
                """Attention for one head pair and one 512-wide q-block."""
                q0, q1 = Qb * 512, (Qb + 1) * 512
                ktmax = 4 * (Qb + 1)
                psZs = []
                for hh in range(2):
                    psZ_h = psB.tile(
                        [65, 512], FP, tag="ZB", name=f"psZ_{pair}_{Qb}_{hh}"
                    )
                    psZs.append(psZ_h)
                def emit_z(kt, e_t, r):
                    for hh in range(2):
                        mm(
                            psZs[hh][:, r:512],
                            v_sb[:, kt, 2 * pair + hh, :],
                            e_t[:, hh, r:512],
                            start=(kt == 0),
                            stop=(kt == ktmax - 1),
                        )

                pending = None  # (kt, e_t, r) -- z emitted one kt behind
                for kt in range(ktmax):
                    # diagonal k-tiles: q-columns < r are fully masked, so
                    # scores/exp/z are all computed on [r:512] only
                    diag = kt >= 4 * Qb
                    r = (kt - 4 * Qb) * 128 if diag else 0
                    # fp32r needs N>=256 for full rate; a 128-wide slice
                    # would run at 4 cyc/row and win nothing
                    r = min(r, 256)
                    # both heads' scores in one 2-bank PSUM tile; the two
                    # K=64 matmuls hit disjoint PE row groups and overlap
                    psS = psA.tile(
                        [128, 2, 512], FP, tag="A", name=f"psS_{pair}_{Qb}_{kt}"
                    )
                    for hh in range(2):
                        po = hh * 64
                        mm(
                            psS[:, hh, r:512],
                            kt_sb[po : po + 64, pair, kt * 128 : (kt + 1) * 128],
                            qt_sb[po : po + 64, pair, q0 + r : q1],
                            start=True,
                            stop=True,
                        )
                    e_t = etp.tile(
                        [128, 2, 512], MT, tag="et", name=f"et_{pair}_{Qb}_{kt}"
                    )
                    nc.scalar.activation(
                        e_t[:, :, r:512], psS[:, :, r:512], AF.Exp
                    )
                    if diag:
                        # causal 0/1 mask as a post-exp multiply, trimmed to
                        # the columns that actually contain masked elements:
                        # [r, (j+1)*128) for diag offset j (cols beyond that
                        # see no k > q in this tile)
                        j = kt - 4 * Qb
                        c1 = (j + 1) * 128
                        for hh in range(2):
                            nc.vector.tensor_mul(
                                e_t[:, hh, r:c1],
                                e_t[:, hh, r:c1],
                                masks_sb[:, j, r:c1],
                            )
                    if not zpipe:
                        emit_z(kt, e_t, r)
                    else:
                        if pending is not None:
                            emit_z(*pending)
                        pending = (kt, e_t, r)
                if zpipe:
                    emit_z(*pending)
                rds = []
                for hh in range(2):
                    rd_h = rdpool.tile(
                        [1, 512], MT, tag="rd", name=f"rd_{pair}_{Qb}_{hh}"
                    )
                    rds.append(rd_h)
                    if recip_ln:
                        # 1/d = exp(-ln d) on ACT: both funcs live in the
                        # natural_log_exp_and_others table (no table switch),
                        # and ACT writes rounded fp32r for the bc matmul.
                        # InstReciprocal on a [1,512] single-partition tile
                        # costs 3.3us on DVE; this is ~2x430ns on ACT.
                        ln_h = rdpool.tile(
                            [1, 512], FP, tag="rd", name=f"ln_{pair}_{Qb}_{hh}"
                        )
                        nc.scalar.activation(
                            ln_h, psZs[hh][64:65, :], AF.Ln
                        )
                        nc.scalar.activation(
                            rd_h, ln_h, AF.Exp, scale=-1.0
                        )
                    else:
                        nc.vector.reciprocal(rd_h, psZs[hh][64:65, :])
                if bcast_dma:
                    # broadcast 1/denom to 128 partitions via a DRAM bounce:
                    # SBUF row -> DRAM, then DRAM -> 128 partitions with a
                    # stride-0 partition read (no PE work, no PSUM bank)
                    bc = bcp.tile([128, 2, 512], MT, tag="bcs",
                                  name=f"bc_{pair}_{Qb}")
                    for hh in range(2):
                        nc.sync.dma_start(
                            out=bass.AP(
                                tensor=rdbc,
                                offset=(((pair * 4 + Qb) * 2 + hh) * 512) * 4,
                                ap=[[1, 512]],
                            ),
                            in_=rds[hh][:, :],
                        )
                        nc.sync.dma_start(
                            out=bc[:, hh, :],
                            in_=bass.AP(
                                tensor=rdbc,
                                offset=(((pair * 4 + Qb) * 2 + hh) * 512) * 4,
                                ap=[[0, 128], [1, 512]],
                            ),
                        )
                    bch = [bc[0:64, 0, :], bc[64:128, 1, :]]
                else:
                    # broadcast 1/denom to a stacked [128, 512] PSUM tile via
                    # two K=1 matmuls against selector rows
                    bc = psB.tile([128, 512], FP, tag="ZB", name=f"bc_{pair}_{Qb}")
                    mm(bc, sel_sb[:, 0, :], rds[0], start=True, stop=False)
                    mm(bc, sel_sb[:, 1, :], rds[1], start=False, stop=True)
                    bch = [bc[0:64, :], bc[64:128, :]]
                # stage z into znp (SBUF), then normalize in place against the
                # broadcast -- at most one PSUM operand per DVE op
                nc.vector.tensor_copy(znp[0:64, pair, Qb, :], psZs[0][0:64, :])
                nc.vector.tensor_mul(
                    znp[0:64, pair, Qb, :],
                    znp[0:64, pair, Qb, :],
                    bch[0],
                )
                nc.vector.tensor_copy(znp[64:128, pair, Qb, :], psZs[1][0:64, :])
                nc.vector.tensor_mul(
                    znp[64:128, pair, Qb, :],
                    znp[64:128, pair, Qb, :],
                    bch[1],
                )

            # ---- order=3: fine-grained quanta + deferred normalization ----
            def qk_quantum(sb, x_t, qt_sb, kt_sb, pair, which):
                w_sb = wq_sb if which == 0 else wk_sb
                ps = psP.tile(
                    [128, 512], FP, tag="pp", name=f"psqk_{sb}_{pair}_{which}"
                )
                for c in range(8):
                    mm(
                        ps,
                        w_sb[:, pair, c, :],
                        x_t[:, c, :],
                        start=(c == 0),
                        stop=(c == 7),
                    )
                t_sb = qt_sb if which == 0 else kt_sb
                dst = t_sb[:, pair, sb * 512 : (sb + 1) * 512]
                if qk_bias:
                    b_sb = bq_sb if which == 0 else bk_sb
                    nc.scalar.activation(
                        dst, ps, AF.Identity, bias=b_sb[:, pair : pair + 1]
                    )
                else:
                    nc.vector.tensor_copy(dst, ps)

            def v_quantum(sb, x_t, v_sb, stl):
                st = sb * 4 + stl
                psV = psP.tile([128, 256], FP, tag="pp", name=f"psV3_{st}")
                for c in range(8):
                    mm(
                        psV,
                        x_t[:, c, stl * 128 : (stl + 1) * 128],
                        wv_sb[:, c, :],
                        start=(c == 0),
                        stop=(c == 7),
                    )
                vv = psV.rearrange("p (h d) -> p h d", h=4)
                if v_bias:
                    nc.vector.tensor_add(v_sb[:, st, :, 0:64], vv, bvbc_sb)
                else:
                    nc.vector.tensor_copy(v_sb[:, st, :, 0:64], vv)

            def out_quanta_for(Qb, znp):
                qs = []
                for stl in range(4):
                    st = Qb * 4 + stl
                    box = {}

                    def q_mk(st, Db, box):
                        def q():
                            if Db == 0:
                                box["t"] = ostp.tile(
                                    [128, D], FP, tag="ost", name=f"ost3_{st}"
                                )
                            ost_t = box["t"]
                            psO = psP.tile(
                                [128, 512], FP, tag="pp", name=f"psO3_{st}_{Db}"
                            )
                            for pair in range(2):
                                mm(
                                    psO,
                                    znp[:, pair, Qb, (st % 4) * 128 : (st % 4) * 128 + 128],
                                    wo_sb[:, pair, Db * 512 : (Db + 1) * 512],
                                    start=(pair == 0),
                                    stop=(pair == 1),
                                )
                            nc.vector.tensor_copy(
                                ost_t[:, Db * 512 : (Db + 1) * 512], psO
                            )
                            if Db == 1:
                                nc.sync.dma_start(
                                    out=out[st * 128 : (st + 1) * 128, :],
                                    in_=ost_t,
                                )

                        return q

                    qs += [q_mk(st, 0, box), q_mk(st, 1, box)]
                return qs

            def emit_attn3(pair, Qb, qt_sb, kt_sb, v_sb, znp, fillers, due, inject):
                """Attention with zpipe depth 2, paced filler quanta pulled
                into the kt loop, and normalization of the PREVIOUS pair
                injected after kt 1 (so its bc matmul never stalls PE)."""
                q0, q1 = Qb * 512, (Qb + 1) * 512
                ktmax = 4 * (Qb + 1)
                # psZ allocated lazily at the first z-matmul: the PREVIOUS
                # pair's norm (injected at kt==1, before z0 at kt==2) must
                # have its psZ reads registered before these slots are reused
                psZs = []

                def emit_z(kt, e_t, r):
                    if not psZs:
                        psZs.extend(
                            psB.tile(
                                [65, 512], FP, tag="zp",
                                name=f"psZ3_{pair}_{Qb}_{hh}",
                            )
                            for hh in range(2)
                        )
                    for hh in range(2):
                        mm(
                            psZs[hh][:, r:512],
                            v_sb[:, kt, 2 * pair + hh, :],
                            e_t[:, hh, r:512],
                            start=(kt == 0),
                            stop=(kt == ktmax - 1),
                        )

                pend = []
                for kt in range(ktmax):
                    diag = kt >= 4 * Qb
                    r = min((kt - 4 * Qb) * 128 if diag else 0, 256)
                    psS = psA.tile(
                        [128, 2, 512], FP, tag="A", name=f"psS3_{pair}_{Qb}_{kt}"
                    )
                    for hh in range(2):
                        po = hh * 64
                        mm(
                            psS[:, hh, r:512],
                            kt_sb[po : po + 64, pair, kt * 128 : (kt + 1) * 128],
                            qt_sb[po : po + 64, pair, q0 + r : q1],
                            start=True,
                            stop=True,
                        )
                    e_t = etp.tile(
                        [128, 2, 512], MT, tag="et", name=f"et3_{pair}_{Qb}_{kt}"
                    )
                    nc.scalar.activation(e_t[:, :, r:512], psS[:, :, r:512], AF.Exp)
                    if diag:
                        j = kt - 4 * Qb
                        c1 = (j + 1) * 128
                        eng = nc.gpsimd if gmask else nc.vector
                        for hh in range(2):
                            eng.tensor_mul(
                                e_t[:, hh, r:c1],
                                e_t[:, hh, r:c1],
                                masks_sb[:, j, r:c1],
                            )
                    pend.append((kt, e_t, r))
                    if len(pend) > 2:
                        emit_z(*pend.pop(0))
                    if kt == 1 and inject:
                        for f in inject:
                            f()
                        inject.clear()
                    for _ in range(due()):
                        if fillers:
                            fillers.popleft()()
                for p in pend:
                    emit_z(*p)
                rds = []
                for hh in range(2):
                    rd_h = rdpool.tile(
                        [1, 512], MT, tag="rd", name=f"rd3_{pair}_{Qb}_{hh}"
                    )
                    ln_h = rdpool.tile(
                        [1, 512], FP, tag="rd", name=f"ln3_{pair}_{Qb}_{hh}"
                    )
                    nc.scalar.activation(ln_h, psZs[hh][64:65, :], AF.Ln)
                    nc.scalar.activation(rd_h, ln_h, AF.Exp, scale=-1.0)
                    rds.append(rd_h)
                # stage unnormalized z into znp NOW -- this releases the psZ
                # PSUM banks quickly so the next pair's z accumulation never
                # stalls on the pool slot
                nc.vector.tensor_copy(znp[0:64, pair, Qb, :], psZs[0][0:64, :])
                nc.vector.tensor_copy(
                    znp[64:128, pair, Qb, :], psZs[1][0:64, :]
                )
                if bcast_dma:
                    # broadcast 1/denom via a DRAM bounce (no PE, no PSUM);
                    # in flight while the next pair's attention runs
                    bc = bcp.tile(
                        [128, 2, 512], MT, tag="bcs", name=f"bc3_{pair}_{Qb}"
                    )
                    for hh in range(2):
                        off = ((pair * 4 + Qb) * 2 + hh) * 512
                        nc.sync.dma_start(
                            out=bass.AP(tensor=rdbc, offset=off, ap=[[1, 512]]),
                            in_=rds[hh][:, :],
                        )
                        nc.sync.dma_start(
                            out=bc[:, hh, :],
                            in_=bass.AP(
                                tensor=rdbc, offset=off, ap=[[0, 128], [1, 512]]
                            ),
                        )

                def norm():
                    if bcast_dma:
                        bch = [bc[0:64, 0, :], bc[64:128, 1, :]]
                    else:
                        bcp_t = psP.tile(
                            [128, 512], FP, tag="pp", name=f"bc3_{pair}_{Qb}"
                        )
                        mm(bcp_t, sel_sb[:, 0, :], rds[0], start=True, stop=False)
                        mm(bcp_t, sel_sb[:, 1, :], rds[1], start=False, stop=True)
                        bch = [bcp_t[0:64, :], bcp_t[64:128, :]]
                    nc.vector.tensor_mul(
                        znp[0:64, pair, Qb, :],
                        znp[0:64, pair, Qb, :],
                        bch[0],
                    )
                    nc.vector.tensor_mul(
                        znp[64:128, pair, Qb, :],
                        znp[64:128, pair, Qb, :],
                        bch[1],
                    )

                return norm

            def emit_out(st, znp):
                """Output projection + store for one 128-row s-tile."""
                Qb, soff = st // 4, (st % 4) * 128
                ost_t = ostp.tile([128, D], FP, tag="ost", name=f"ost_{st}")
                for Db in range(2):
                    psO = psB.tile(
                        [128, 512], FP, tag="ZB", name=f"psO_{st}_{Db}"
                    )
                    for pair in range(2):
                        mm(
                            psO,
                            znp[:, pair, Qb, soff : soff + 128],
                            wo_sb[:, pair, Db * 512 : (Db + 1) * 512],
                            start=(pair == 0),
                            stop=(pair == 1),
                        )
                    if Db == 0:
                        nc.vector.tensor_copy(
                            ost_t[:, Db * 512 : (Db + 1) * 512], psO
                        )
                    else:
                        nc.scalar.activation(
                            ost_t[:, Db * 512 : (Db + 1) * 512], psO, AF.Copy
                        )
                # issue from the DVE sequencer: the store's wait target is
                # the DVE copy that just ran there, so it can't head-of-line
                # block the SP stream that issues x loads
                nc.sync.dma_start(
                    out=out[st * 128 : (st + 1) * 128, :], in_=ost_t
                )

            for _rep in range(reps):
                qt_sb = qk.tile([128, 2, S], MT, tag="qt")
                kt_sb = qk.tile([128, 2, S], MT, tag="kt")
                v_sb = vp.tile([128, 16, 4, 65], MT, tag="v")
                znp = zp.tile([128, 2, 4, 512], MT, tag="zn")
                # ones column of V' (written once; proj fills the rest)
                nc.vector.memset(v_sb[:, :, :, 64:65].bitcast(FP), 1.0)
                inject3 = []

                # interleaved emission: attention for q-block Qb only needs
                # projections of s-blocks <= Qb, so proj(sb) / attn(Qb=sb) /
                # out-proj(Qb=sb) alternate -- projection PE work fills the
                # gaps while ScalarE grinds through the exps
                x_tiles = {}
                for sb in range(4):
                    if sb not in x_tiles:
                        x_tiles[sb] = emit_x(
                            sb, x_pre=x0_t if (_rep == 0 and sb == 0) else None
                        )
                    x_t = x_tiles[sb]
                    # prefetch x blocks ahead
                    for ahead in range(1, xpf + 1):
                        if sb + ahead <= 3 and sb + ahead not in x_tiles:
                            x_tiles[sb + ahead] = emit_x(sb + ahead)
                    if order == 3:
                        import collections

                        fillers = collections.deque()
                        if sb == 0:
                            # dense startup: only pair0's Q/K -- attention
                            # starts immediately; V and pair1 proj flow in
                            # as early fillers
                            qk_quantum(0, x_t, qt_sb, kt_sb, 0, 0)
                            qk_quantum(0, x_t, qt_sb, kt_sb, 0, 1)
                            if _rep == 0:
                                emit_const_group_b()
                            for stl in range(4):
                                fillers.append(
                                    lambda stl=stl, x0=x_t: v_quantum(
                                        0, x0, v_sb, stl
                                    )
                                )
                            for which in (0, 1):
                                fillers.append(
                                    lambda which=which, x0=x_t: qk_quantum(
                                        0, x0, qt_sb, kt_sb, 1, which
                                    )
                                )
                        if sb < 3:
                            if sb + 1 not in x_tiles:
                                x_tiles[sb + 1] = emit_x(sb + 1)
                            x_next = x_tiles[sb + 1]
                            for pair in (0, 1):
                                for which in (0, 1):
                                    fillers.append(
                                        lambda pair=pair, which=which: qk_quantum(
                                            sb + 1, x_next, qt_sb, kt_sb, pair, which
                                        )
                                    )
                            for stl in range(4):
                                fillers.append(
                                    lambda stl=stl: v_quantum(
                                        sb + 1, x_next, v_sb, stl
                                    )
                                )
                        if sb >= 1:
                            fillers.extend(out_quanta_for(sb - 1, znp))
                        n0 = len(fillers)
                        total_kts = 2 * 4 * (sb + 1)
                        ktc = {"v": 0}

                        def due(n0=n0, total_kts=total_kts, ktc=ktc,
                                fillers=fillers, nosupp=(sb == 0)):
                            """Called once per kt; returns how many filler
                            quanta to pull to stay on pace (pace/4 quanta per
                            kt, front-loaded). No pulls during the first two
                            kts of a pair -- the inject (deferred norm of the
                            previous pair) lands at kt==1 and out-proj
                            fillers depend on its znp. sb==0 has no out-proj
                            fillers, so it pulls from kt 0 (the V tiles must
                            land before the first z matmuls)."""
                            ktc["v"] += 1
                            k_in_pair = (ktc["v"] - 1) % (total_kts // 2)
                            if not nosupp and k_in_pair < 2:
                                return 0
                            target = min(
                                n0,
                                (ktc["v"] * n0 * pace + 4 * total_kts - 1)
                                // (4 * total_kts),
                            )
                            done = n0 - len(fillers)
                            return max(0, target - done)

                        norm_p0 = emit_attn3(
                            0, sb, qt_sb, kt_sb, v_sb, znp,
                            fillers, due, inject3,
                        )
                        inject3 = [norm_p0]
                        norm_p1 = emit_attn3(
                            1, sb, qt_sb, kt_sb, v_sb, znp,
                            fillers, due, inject3,
                        )
                        inject3 = [norm_p1]
                        while fillers:
                            fillers.popleft()()
                        if sb == 3:
                            for f in inject3:
                                f()
                            inject3 = []
                            for q in out_quanta_for(3, znp):
                                q()
                        continue
                    if order == 0:
                        emit_qk(sb, x_t, qt_sb, kt_sb)
                        emit_v(sb, x_t, v_sb)
                        if _rep == 0 and sb == 0:
                            emit_const_group_b()
                        if upto >= 2:
                            for pair in range(2):
                                emit_attn(pair, sb, qt_sb, kt_sb, v_sb, znp)
                    elif order == 2:
                        # out-proj of the previous s-block emitted between
                        # the two attention passes as mid-segment PE filler
                        emit_qk(sb, x_t, qt_sb, kt_sb)
                        emit_v(sb, x_t, v_sb)
                        if _rep == 0 and sb == 0:
                            emit_const_group_b()
                        if upto >= 2:
                            emit_attn(0, sb, qt_sb, kt_sb, v_sb, znp)
                        if upto >= 3 and sb > 0:
                            for st in range(4 * sb - 4, 4 * sb):
                                emit_out(st, znp)
                        if upto >= 2:
                            emit_attn(1, sb, qt_sb, kt_sb, v_sb, znp)
                    else:
                        # pair-1 projections emitted between the two
                        # attention passes: PE fills attention(pair0)'s
                        # ScalarE-bound stretch with projection matmuls
                        emit_qk(sb, x_t, qt_sb, kt_sb, pairs=(0,))
                        emit_v(sb, x_t, v_sb)
                        if _rep == 0 and sb == 0:
                            emit_const_group_b()
                        if upto >= 2:
                            emit_attn(0, sb, qt_sb, kt_sb, v_sb, znp)
                        emit_qk(sb, x_t, qt_sb, kt_sb, pairs=(1,))
                        if upto >= 2:
                            emit_attn(1, sb, qt_sb, kt_sb, v_sb, znp)
                    if upto >= 3 and (order != 2 or sb == 3):
                        for st in range(4 * sb, 4 * sb + 4):
                            emit_out(st, znp)

    return _hook_wait_split(nc)



# ---------------------------------------------------------------------------
# Persistent PJRT runner (mirrors run_bass_via_pjrt, but keeps the jitted
# callable so repeated kernel() calls don't recompile)
# ---------------------------------------------------------------------------
class _Runner:
    def __init__(self, nc):
        import jax
        import jax.numpy as jnp  # noqa: F401
        import numpy as _np
        from jax.experimental.shard_map import shard_map
        from jax.sharding import Mesh, PartitionSpec
        import concourse.mybir as mybir
        from concourse.bass2jax import (
            _bass_exec_p,
            install_neuronx_cc_hook,
            partition_id_tensor,
        )

        install_neuronx_cc_hook()
        self.jax = jax
        pname = nc.partition_id_tensor.name if nc.partition_id_tensor else None
        in_names, out_names, out_avals, zero_outs = [], [], [], []
        for alloc in nc.m.functions[0].allocations:
            if not isinstance(alloc, mybir.MemoryLocationSet):
                continue
            name = alloc.memorylocations[0].name
            if alloc.kind == "ExternalInput":
                if name == pname:
                    continue
                in_names.append(name)
            elif alloc.kind == "ExternalOutput":
                shape = tuple(alloc.tensor_shape)
                dtype = mybir.dt.np(alloc.dtype)
                out_names.append(name)
                out_avals.append(jax.core.ShapedArray(shape, dtype))
                zero_outs.append(_np.zeros(shape, dtype))
        self.in_names, self.out_names = list(in_names), list(out_names)
        self.out_avals, self.zero_outs = out_avals, zero_outs
        n_params, n_outs = len(in_names), len(out_names)
        self.n_params = n_params
        all_names = in_names + out_names
        if pname is not None:
            all_names = all_names + [pname]

        def _body(*args):
            operands = list(args)
            if pname is not None:
                operands.append(partition_id_tensor())
            outs = _bass_exec_p.bind(
                *operands,
                out_avals=tuple(out_avals),
                in_names=tuple(all_names),
                out_names=tuple(out_names),
                lowering_input_output_aliases=(),
                sim_require_finite=True,
                sim_require_nnan=True,
                nc=nc,
            )
            return tuple(outs)

        devices = jax.devices()[:NCORES]
        mesh = Mesh(np.asarray(devices), ("core",))
        in_specs = (PartitionSpec("core"),) * (n_params + n_outs)
        out_specs = (PartitionSpec("core"),) * n_outs
        self.fn = jax.jit(
            shard_map(
                _body,
                mesh=mesh,
                in_specs=in_specs,
                out_specs=out_specs,
                check_rep=False,
            ),
            donate_argnums=tuple(range(n_params, n_params + n_outs)),
            keep_unused=True,
        )

    def device_put_inputs(self, concat_in):
        return [self.jax.device_put(a) for a in concat_in]

    def time_exec(self, dev_in, iters=8):
        """Repeat execution with device-resident inputs; the previous call's
        (donated, fully-overwritten) outputs serve as the next call's output
        buffers, so nothing moves over the axon tunnel."""
        import time as _time

        zeros = [
            np.zeros((NCORES * z.shape[0], *z.shape[1:]), z.dtype)
            for z in self.zero_outs
        ]
        r = self.fn(*dev_in, *zeros)
        self.jax.block_until_ready(r)
        times = []
        for _ in range(iters):
            t0 = _time.perf_counter()
            r = self.fn(*dev_in, *r)
            self.jax.block_until_ready(r)
            times.append(_time.perf_counter() - t0)
        return times

    def concat_inputs(self, in_maps):
        return [
            np.concatenate([in_maps[c][n] for c in range(NCORES)], axis=0)
            for n in self.in_names
        ]

    def run_concat(self, concat_in):
        zeros = [
            np.zeros((NCORES * z.shape[0], *z.shape[1:]), z.dtype)
            for z in self.zero_outs
        ]
        outs = self.fn(*concat_in, *zeros)
        outs = [np.asarray(o) for o in outs]
        return outs

    def run(self, in_maps):
        outs = self.run_concat(self.concat_inputs(in_maps))
        per_core = []
        for c in range(NCORES):
            m = {}
            for i, n in enumerate(self.out_names):
                shp = self.out_avals[i].shape
                m[n] = outs[i].reshape(NCORES, *shp)[c]
            per_core.append(m)
        return per_core


def _round_tf32(a):
    """Round fp32 -> TF32 (10-bit mantissa, RNE) so device-side fp32r
    consumers see pre-rounded values."""
    u = np.ascontiguousarray(a, dtype=np.float32).view(np.uint32)
    r = (u + np.uint32(0x1000) + ((u >> np.uint32(13)) & np.uint32(1))) & np.uint32(0xFFFFE000)
    return r.view(np.float32)


def _make_masks():
    """0/1 multiplicative causal masks for the 4 diagonal k-tile offsets."""
    m = np.ones((4, 128, 512), dtype=np.float32)
    for r in range(4):
        p = np.arange(128)[:, None]
        j = np.arange(512)[None, :]
        m[r][p + 128 * r > j] = 0.0
    return _round_tf32(m)


def _prep_core_inputs(inputs):
    """Shard + repack the full problem inputs into per-core input maps."""
    x = np.asarray(inputs["normalized_resid_pre"], dtype=np.float32)
    W_Q = np.asarray(inputs["W_Q"], dtype=np.float32)
    W_K = np.asarray(inputs["W_K"], dtype=np.float32)
    W_V = np.asarray(inputs["W_V"], dtype=np.float32)
    W_O = np.asarray(inputs["W_O"], dtype=np.float32)
    b_Q = np.asarray(inputs["b_Q"], dtype=np.float32)
    b_K = np.asarray(inputs["b_K"], dtype=np.float32)
    b_V = np.asarray(inputs["b_V"], dtype=np.float32)

    scale = np.float32(1.0 / np.sqrt(HD))
    masks = _make_masks()
    sel = np.zeros((2, 128), dtype=np.float32)
    sel[0, 0:64] = 1.0
    sel[1, 64:128] = 1.0

    in_maps = []
    for c in range(NCORES):
        b, g = c // 4, c % 4
        hs = [4 * g + i for i in range(HPC)]
        xTb = _round_tf32(np.ascontiguousarray(x[b].T))  # [D, S]
        wq_p = np.zeros((2, D, 128), dtype=np.float32)
        wk_p = np.zeros((2, D, 128), dtype=np.float32)
        wo_p = np.zeros((2, 128, D), dtype=np.float32)
        bq_p = np.zeros((2, 128), dtype=np.float32)
        bk_p = np.zeros((2, 128), dtype=np.float32)
        for pr in range(2):
            h0, h1 = hs[2 * pr], hs[2 * pr + 1]
            wq_p[pr, :, 0:64] = W_Q[h0] * scale
            wq_p[pr, :, 64:128] = W_Q[h1] * scale
            wk_p[pr, :, 0:64] = W_K[h0]
            wk_p[pr, :, 64:128] = W_K[h1]
            wo_p[pr, 0:64, :] = W_O[h0]
            wo_p[pr, 64:128, :] = W_O[h1]
            bq_p[pr, 0:64] = b_Q[h0] * scale
            bq_p[pr, 64:128] = b_Q[h1] * scale
            bk_p[pr, 0:64] = b_K[h0]
            bk_p[pr, 64:128] = b_K[h1]
        wv_p = np.concatenate([W_V[h] for h in hs], axis=1)  # [D, 256]
        wq_p, wk_p, wv_p, wo_p = (
            _round_tf32(wq_p),
            _round_tf32(wk_p),
            _round_tf32(wv_p),
            _round_tf32(wo_p),
        )
        bv_p = np.concatenate([b_V[h] for h in hs], axis=0)  # [256]
        in_maps.append(
            {
                "xt": xTb,
                "wq": wq_p,
                "wk": wk_p,
                "wv": np.ascontiguousarray(wv_p),
                "wo": wo_p,
                "bq": bq_p,
                "bk": bk_p,
                "bv": np.ascontiguousarray(bv_p),
                "masks": masks,
                "sel": sel,
            }
        )
    return in_maps


def _get_state(qk_bias=True, v_bias=True):
    key = (qk_bias, v_bias)
    if key not in _STATE:
        _STATE[key] = _Runner(
            _build_nc(qk_bias=qk_bias, v_bias=v_bias, order=3)
        )
    return _STATE[key]


def kernel(**inputs):
    need_qk_bias = bool(
        np.any(np.asarray(inputs["b_Q"])) or np.any(np.asarray(inputs["b_K"]))
    )
    need_v_bias = bool(np.any(np.asarray(inputs["b_V"])))
    st = _get_state(qk_bias=need_qk_bias, v_bias=need_v_bias)
    in_maps = _prep_core_inputs(inputs)
    per_core = st.run(in_maps)
    b_O = np.asarray(inputs["b_O"], dtype=np.float32)
    out = np.zeros((B, S, D), dtype=np.float32)
    for c in range(NCORES):
        out[c // 4] += per_core[c]["out"]
    out += b_O[None, None, :]
    return out

